# revision 1
# baseline (speedup 1.0000x reference)
"""Trainium2 Bass kernel for causal MHA (b=2, n=4096, d_model=768, 12 heads).

Sharding: 8 cores = 2 batches x 4 head-groups (3 heads each).
Each core:
  - receives its batch's Q/K/V pre-transposed ([768, n], d_model on rows)
    plus its head-group's weight slices (also pre-transposed on host).
  - projects qT/kT ([64, n] per head, head dim on partitions) and
    v ([n, 64] per head, tokens on partitions) on-chip.
  - computes scoresT[k, q] = kT^T @ qT tile-by-tile (128 keys x <=512
    queries, skipping the fully-masked left part of diagonal tiles),
    exponentiates (no max-subtraction: scores ~ N(0,1), fp32 exp is safe),
    masks the causal boundary block with a precomputed 0/1 mask, and
    accumulates outT_aug[65, q] += [v | ones]^T @ P in PSUM.  Row 64 is
    the softmax denominator; division is folded into the PSUM->SBUF copy.
  - applies the output projection with its w_o row-slice; host sums the
    4 partial outputs per batch (row-parallel linear unshard).

Weight-column host layout packs the six 64-wide q/k heads into three full
128-row M-blocks ([q0;q1], [q2;k2], [k0;k1]); k2 is then DMA-copied to
partitions 0-63 of a fourth block so every head's scores matmul sees its
qT and kT at the same partition base (a matmul constraint).  The same
DMA partition-shift turns outT into [h0;h1] + [h2] so the output
projection contracts in 2 chunks instead of 3.
"""

import sys

for _p in ("/opt/trn_rl_repo",):
    if _p not in sys.path:
        sys.path.insert(0, _p)

import numpy as np
import ml_dtypes

import concourse.bass as bass  # noqa: F401  (registers engine classes)
import concourse.tile as tile
from concourse import bacc, mybir
import concourse.bass_utils as bass_utils

P = 128
D_MODEL = 768
KO = D_MODEL // P  # 6 contraction chunks of 128
N_HEADS = 12
D_K = 64
N_CORES = 8
H_LOCAL = 3  # heads per core
D_LOCAL = H_LOCAL * D_K  # 192
B = 2
N_TOKENS = 4096
NQ = 512  # query-chunk size (one PSUM bank of fp32)
NT = 256  # token chunk for q/k projection

F32 = mybir.dt.float32
BF16 = mybir.dt.bfloat16
F32R = mybir.dt.float32r


def _mm(ap, flavor):
    """View an fp32 AP as the matmul input dtype."""
    if flavor == "f32r":
        return ap.bitcast(F32R)
    return ap


def build_nc(n=N_TOKENS, mm="bf16", dt_x=BF16, dt_pt=BF16, dt_acc=BF16):
    assert n % NQ == 0 and n % NT == 0 and n % P == 0
    nc = bacc.Bacc("TRN2", target_bir_lowering=False, debug=False,
                   num_devices=N_CORES)

    qt_d = nc.dram_tensor("qt", [D_MODEL, n], dt_x, kind="ExternalInput")
    kt_d = nc.dram_tensor("kt", [D_MODEL, n], dt_x, kind="ExternalInput")
    vt_d = nc.dram_tensor("vt", [D_MODEL, n], dt_x, kind="ExternalInput")
    wqk_d = nc.dram_tensor("wqk", [D_MODEL, 2 * D_LOCAL], dt_x,
                           kind="ExternalInput")
    wv_d = nc.dram_tensor("wv", [D_MODEL, D_LOCAL], dt_x, kind="ExternalInput")
    wo_d = nc.dram_tensor("wo", [D_LOCAL, D_MODEL], dt_x, kind="ExternalInput")
    cm_d = nc.dram_tensor("cmask", [P, P], dt_pt, kind="ExternalInput")
    y_d = nc.dram_tensor("y", [n, D_MODEL], F32, kind="ExternalOutput")

    qt_r = qt_d.ap().rearrange("(ko ki) t -> ki ko t", ki=P)
    kt_r = kt_d.ap().rearrange("(ko ki) t -> ki ko t", ki=P)
    vt_r = vt_d.ap().rearrange("(ko ki) t -> ki ko t", ki=P)
    wqk_r = wqk_d.ap().rearrange("(ko ki) m -> ki ko m", ki=P)
    wv_r = wv_d.ap().rearrange("(ko ki) m -> ki ko m", ki=P)

    TCH = n // NT       # q/k projection token chunks
    TB = n // P         # 128-token blocks
    QCH = n // NQ       # query chunks
    KB_PER_Q = NQ // P  # key blocks per query chunk (4)

    # Host weight-column order: [q0 q1 | q2 k2 | k0 k1] -> 3 full M-blocks.
    # qkT_sb blk3[0:64] is a DMA-shifted copy of k2 (blk1[64:128]).
    q_loc = {0: (0, 0), 1: (64, 0), 2: (0, 1)}
    k_loc = {0: (0, 2), 1: (64, 2), 2: (0, 3)}

    with tile.TileContext(nc) as tc:
        with tc.tile_pool(name="const", bufs=1) as cpool, \
             tc.tile_pool(name="persist", bufs=1) as ppool, \
             tc.tile_pool(name="xqk", bufs=3) as xpool, \
             tc.tile_pool(name="xv", bufs=2) as xvpool, \
             tc.tile_pool(name="pt", bufs=6) as ptpool, \
             tc.tile_pool(name="ysb", bufs=2) as ypool, \
             tc.tile_pool(name="rcp", bufs=2) as rpool, \
             tc.tile_pool(name="ot", bufs=2) as otpool, \
             tc.tile_pool(name="dbounce", bufs=2, space="DRAM") as dpool, \
             tc.tile_pool(name="pp_proj", bufs=2, space="PSUM") as pp_proj, \
             tc.tile_pool(name="pp_sc", bufs=3, space="PSUM") as pp_sc, \
             tc.tile_pool(name="pp_out", bufs=1, space="PSUM") as pp_out:

            # ---- constants ----
            wqk_sb = cpool.tile([P, KO, 2 * D_LOCAL], dt_x)
            nc.sync.dma_start(wqk_sb[:], wqk_r)
            wv_sb = cpool.tile([P, KO, D_LOCAL], dt_x)
            nc.sync.dma_start(wv_sb[:], wv_r)
            # w_o rows: chunk0 = dims of h0,h1 (128 rows), chunk1 = h2 (64)
            wo_sb = cpool.tile([P, 2, D_MODEL], dt_x)
            nc.sync.dma_start(wo_sb[:, 0, :], wo_d.ap()[0:P, :])
            nc.sync.dma_start(wo_sb[0:64, 1, :], wo_d.ap()[P:D_LOCAL, :])
            cm_sb = cpool.tile([P, P], dt_pt)
            nc.sync.dma_start(cm_sb[:], cm_d.ap())

            # ---- persistent activations ----
            qkT_sb = ppool.tile([P, 4, n], dt_acc)
            v_sb = ppool.tile([P, TB, H_LOCAL, 66], dt_acc)
            outT_sb = ppool.tile([P, 2, n], dt_acc)
            nc.vector.memset(v_sb[:, :, :, 64:65], 1.0)

            # ---- q/k projections (transposed layout, 3 packed M-blocks) ----
            for t in range(TCH):
                xq = xpool.tile([P, KO, NT], dt_x, tag="x")
                nc.sync.dma_start(xq[:], qt_r[:, :, t * NT:(t + 1) * NT])
                xk = xpool.tile([P, KO, NT], dt_x, tag="x")
                nc.sync.dma_start(xk[:], kt_r[:, :, t * NT:(t + 1) * NT])
                for blk in range(3):
                    ps = pp_proj.tile([P, NQ], F32, tag="psproj")
                    for ko in range(KO):
                        # blk1 contracts q2 against Q-input and k2 against
                        # K-input: split into two half-partition matmuls.
                        if blk == 1:
                            nc.tensor.matmul(
                                ps[0:64, 0:NT],
                                _mm(wqk_sb[:, ko, 128:192], mm),
                                _mm(xq[:, ko, :], mm),
                                start=(ko == 0), stop=(ko == KO - 1),
                                skip_group_check=True,
                            )
                            nc.tensor.matmul(
                                ps[64:128, 0:NT],
                                _mm(wqk_sb[:, ko, 192:256], mm),
                                _mm(xk[:, ko, :], mm),
                                start=(ko == 0), stop=(ko == KO - 1),
                                skip_group_check=True,
                            )
                        else:
                            x = xq if blk == 0 else xk
                            nc.tensor.matmul(
                                ps[:, 0:NT],
                                _mm(wqk_sb[:, ko, blk * 128:(blk + 1) * 128], mm),
                                _mm(x[:, ko, :], mm),
                                start=(ko == 0), stop=(ko == KO - 1),
                            )
                    nc.vector.tensor_copy(
                        out=qkT_sb[:, blk, t * NT:(t + 1) * NT],
                        in_=ps[:, 0:NT],
                    )
                # Partition-shifted copies so h2's scores matmul sees qT/kT
                # at the same base — and at BOTH bases, so h2 can alternate
                # row-groups and pair with whichever half is free:
                #   blk3[0:64]   = k2 (from blk1[64:128])
                #   blk3[64:128] = q2 (from blk1[0:64])
                nc.sync.dma_start(
                    qkT_sb[0:64, 3, t * NT:(t + 1) * NT],
                    qkT_sb[64:128, 1, t * NT:(t + 1) * NT],
                )
                nc.sync.dma_start(
                    qkT_sb[64:128, 3, t * NT:(t + 1) * NT],
                    qkT_sb[0:64, 1, t * NT:(t + 1) * NT],
                )

            # ---- v projection (token-major layout) ----
            for tb in range(TB):
                xv = xvpool.tile([P, KO, P], dt_x)
                nc.sync.dma_start(xv[:], vt_r[:, :, tb * P:(tb + 1) * P])
                ps = pp_proj.tile([P, NQ], F32, tag="psproj")
                for ko in range(KO):
                    nc.tensor.matmul(
                        ps[:, 0:D_LOCAL],
                        _mm(xv[:, ko, :], mm),
                        _mm(wv_sb[:, ko, :], mm),
                        start=(ko == 0), stop=(ko == KO - 1),
                    )
                for h in range(H_LOCAL):
                    nc.vector.tensor_copy(
                        out=v_sb[:, tb, h, 0:64],
                        in_=ps[:, h * 64:(h + 1) * 64],
                    )

            # ---- causal attention, transposed-score flash style ----
            # Heads are interleaved so the PE runs two concurrent score
            # matmuls on disjoint row-groups: h0 lives at partitions 0-63,
            # h1 at 64-127, h2 alternates base per key-block (its qT/kT are
            # replicated at both bases in blk1/blk3).
            def h2_qk(kb):
                if kb % 2 == 0:
                    return (0, 1), (0, 3)   # q2 @ blk1[0:64], k2' @ blk3[0:64]
                return (64, 3), (64, 1)     # q2' @ blk3[64:128], k2 @ blk1[64:128]

            def qk_for(h, kb):
                if h == 2:
                    return h2_qk(kb)
                return q_loc[h], k_loc[h]

            for j in range(QCH):
                po = [pp_out.tile([P, NQ], F32, tag=f"po{h}", name=f"po{h}")
                      for h in range(H_LOCAL)]
                nkb = KB_PER_Q * j + KB_PER_Q
                for kb2 in range(0, nkb, 2):
                    # pairing order: [s_h0||s_h1](kb2), [s_h2(kb2)||s_h2(kb2+1)],
                    # [s_h0||s_h1](kb2+1)
                    order = [(0, kb2), (1, kb2), (2, kb2), (2, kb2 + 1),
                             (0, kb2 + 1), (1, kb2 + 1)]
                    pts = {}
                    for (h, kb) in order:
                        (qp, qb), (kp, kb_) = qk_for(h, kb)
                        kloc = kb - KB_PER_Q * j
                        off = max(kloc, 0) * P  # masked part of diag tiles
                        psc = pp_sc.tile([P, NQ], F32, tag="psc", name="psc")
                        nc.tensor.matmul(
                            psc[:, off:],
                            _mm(qkT_sb[kp:kp + 64, kb_, kb * P:(kb + 1) * P], mm),
                            _mm(qkT_sb[qp:qp + 64, qb,
                                       j * NQ + off:(j + 1) * NQ], mm),
                            start=True, stop=True,
                        )
                        pt = ptpool.tile([P, NQ], dt_pt, name="pt")
                        nc.scalar.activation(pt[:, off:], psc[:, off:],
                                             mybir.ActivationFunctionType.Exp)
                        if kloc >= 0:
                            nc.vector.tensor_mul(out=pt[:, off:off + P],
                                                 in0=pt[:, off:off + P],
                                                 in1=cm_sb[:])
                        pts[(h, kb)] = (pt, off)
                    for (h, kb) in order:
                        pt, off = pts[(h, kb)]
                        nc.tensor.matmul(
                            po[h][0:65, off:],
                            _mm(v_sb[:, kb, h, 0:65], mm),
                            _mm(pt[:, off:], mm),
                            start=(kb == 0), stop=(kb == nkb - 1),
                        )
                # Copy each po to SBUF right away (frees the PSUM bank for
                # the next chunk), then normalize from SBUF.
                for h in range(H_LOCAL):
                    raw = rpool.tile([65, NQ], F32, tag="raw", name="raw")
                    nc.vector.tensor_copy(out=raw[:], in_=po[h][0:65, :])
                    r1 = rpool.tile([65, NQ], F32, tag="r1", name="r1")
                    nc.vector.reciprocal(r1[64:65, :], raw[64:65, :])
                    db = dpool.tile([1, NQ], F32, name="db")
                    nc.sync.dma_start(db[:], r1[64:65, :])
                    rr = rpool.tile([64, NQ], F32, tag="rr", name="rr")
                    nc.sync.dma_start(rr[:], db[:].to_broadcast((64, NQ)))
                    if h == 1:
                        # h1 lives at partitions 64-127 of outT blk0; DVE
                        # lanes are partition-locked, so write a temp at
                        # base 0 and DMA partition-shift it up.
                        ot = otpool.tile([64, NQ], dt_acc, name="ot")
                        nc.vector.tensor_mul(out=ot[:], in0=raw[0:64, :],
                                             in1=rr[:])
                        nc.sync.dma_start(
                            outT_sb[64:128, 0, j * NQ:(j + 1) * NQ], ot[:])
                    else:
                        dst = outT_sb[0:64, 0 if h == 0 else 1,
                                      j * NQ:(j + 1) * NQ]
                        nc.vector.tensor_mul(out=dst, in0=raw[0:64, :],
                                             in1=rr[:])

            # ---- output projection (K = 128 + 64) ----
            NOC = 2  # 768 = 2 x 384
            NO = D_MODEL // NOC
            for tb in range(TB):
                for oc in range(NOC):
                    ps = pp_proj.tile([P, NQ], F32, tag="psproj")
                    nc.tensor.matmul(
                        ps[:, 0:NO],
                        _mm(outT_sb[:, 0, tb * P:(tb + 1) * P], mm),
                        _mm(wo_sb[:, 0, oc * NO:(oc + 1) * NO], mm),
                        start=True, stop=False,
                    )
                    nc.tensor.matmul(
                        ps[:, 0:NO],
                        _mm(outT_sb[0:64, 1, tb * P:(tb + 1) * P], mm),
                        _mm(wo_sb[0:64, 1, oc * NO:(oc + 1) * NO], mm),
                        start=False, stop=True,
                    )
                    ysb = ypool.tile([P, NO], F32)
                    nc.vector.tensor_copy(out=ysb[:], in_=ps[:, 0:NO])
                    nc.sync.dma_start(
                        y_d.ap()[tb * P:(tb + 1) * P, oc * NO:(oc + 1) * NO],
                        ysb[:],
                    )

    nc.compile()
    return nc


def make_causal_mask_np(dt=np.float32):
    """[128, 128] lower-left keep mask: m[p, f] = 1.0 iff f >= p."""
    f = np.arange(P)[None, :]
    p = np.arange(P)[:, None]
    return (f >= p).astype(np.float32).astype(dt)


def prep_core_inputs(Q, K, V, w_q, w_k, w_v, w_o, core, n=N_TOKENS,
                     np_x=ml_dtypes.bfloat16, np_pt=ml_dtypes.bfloat16):
    """Host-side sharding/layout prep for one core. All fp32 numpy in."""
    b = core // 4
    g = core % 4
    hs = g * D_LOCAL
    scale = 1.0 / np.sqrt(D_K)
    qt = np.ascontiguousarray(Q[b].T).astype(np_x)
    kt = np.ascontiguousarray(K[b].T).astype(np_x)
    vt = np.ascontiguousarray(V[b].T).astype(np_x)
    wql = w_q[hs:hs + D_LOCAL] * scale
    wkl = w_k[hs:hs + D_LOCAL]
    # column order [q0 q1 | q2 k2 | k0 k1] (see build_nc)
    wqk = np.ascontiguousarray(
        np.concatenate([wql[0:128], wql[128:192], wkl[128:192], wkl[0:128]],
                       axis=0).T
    ).astype(np_x)
    wv = np.ascontiguousarray(w_v[hs:hs + D_LOCAL].T).astype(np_x)
    wo = np.ascontiguousarray(w_o[:, hs:hs + D_LOCAL].T).astype(np_x)
    cm = make_causal_mask_np(np_pt)
    return {"qt": qt, "kt": kt, "vt": vt, "wqk": wqk, "wv": wv, "wo": wo,
            "cmask": cm}


_NC_CACHE = {}


def _get_nc(key, **kw):
    if key not in _NC_CACHE:
        _NC_CACHE[key] = build_nc(**kw)
    return _NC_CACHE[key]


KCFG = {"mm": "bf16", "dt_x": BF16, "dt_pt": BF16, "dt_acc": BF16,
        "np_x": ml_dtypes.bfloat16, "np_pt": ml_dtypes.bfloat16}


def kernel(Q, K, V, w_q, w_k, w_v, w_o):
    Q = np.asarray(Q, dtype=np.float32)
    K = np.asarray(K, dtype=np.float32)
    V = np.asarray(V, dtype=np.float32)
    w_q = np.asarray(w_q, dtype=np.float32)
    w_k = np.asarray(w_k, dtype=np.float32)
    w_v = np.asarray(w_v, dtype=np.float32)
    w_o = np.asarray(w_o, dtype=np.float32)

    nc = _get_nc((KCFG["mm"], str(KCFG["dt_x"])),
                 n=N_TOKENS, mm=KCFG["mm"], dt_x=KCFG["dt_x"],
                 dt_pt=KCFG["dt_pt"], dt_acc=KCFG["dt_acc"])
    in_maps = [
        prep_core_inputs(Q, K, V, w_q, w_k, w_v, w_o, c,
                         np_x=KCFG["np_x"], np_pt=KCFG["np_pt"])
        for c in range(N_CORES)
    ]
    res = bass_utils.run_bass_kernel_spmd(nc, in_maps,
                                          core_ids=list(range(N_CORES)))
    out = np.zeros((B, N_TOKENS, D_MODEL), dtype=np.float32)
    for c in range(N_CORES):
        out[c // 4] += res.results[c]["y"]
    return out



# revision 58
# speedup vs baseline: 1.7372x; 1.7372x over previous
"""Trainium2 Bass kernel for causal MHA (b=2, n=4096, d_model=768, 12 heads).

Sharding: 8 cores = 2 batches x 4 head-groups (3 heads each).
Each core:
  - receives its batch's Q/K/V pre-transposed ([768, n], d_model on rows)
    plus its head-group's weight slices (also pre-transposed on host).
  - projects qT/kT ([64, n] per head, head dim on partitions) and
    v ([n, 64] per head, tokens on partitions) on-chip.
  - computes scoresT[k, q] = kT^T @ qT for key-block PAIRS into a 2-bank
    PSUM tile, exponentiates both blocks with ONE wide ACT instruction
    (halves the per-instruction overhead on the bottleneck engine),
    masks the causal boundary blocks, and accumulates
    outT_aug[65, q] += [v | ones]^T @ P in PSUM (row 64 = denominator).
  - normalizes via reciprocal_approx_fast + gpsimd partition_broadcast
    (no DRAM bounce, no 8-cycle/elem reciprocal).
  - applies the output projection with its w_o row-slice; host sums the
    4 bf16 partial outputs per batch (row-parallel linear unshard).

The emission order software-pipelines the phases: q/k/v projection
chunks and the output projection of the previous query chunk are
interleaved between attention pair-groups, so the Scalar engine (the
bottleneck: ~26M exp elements) is kept fed from the first microsecond
and the PE never idles long enough to lose its HAM clock boost.

Weight-column host layout packs the six 64-wide q/k heads into three full
128-row M-blocks ([q0;q1], [q2;k2], [k0;k1]); k2/q2 are then DMA-copied to
a fourth block so every head's scores matmul sees its qT and kT at the
same partition base (a matmul constraint), with h2 replicated at both
bases so it can alternate row-groups per key block and pair with itself.
"""

import sys

for _p in ("/opt/trn_rl_repo",):
    if _p not in sys.path:
        sys.path.insert(0, _p)

import numpy as np
import ml_dtypes

import concourse.bass as bass  # noqa: F401  (registers engine classes)
import concourse.tile as tile
from concourse import bacc, mybir
import concourse.bass_utils as bass_utils

P = 128
D_MODEL = 768
KO = D_MODEL // P  # 6 contraction chunks of 128
N_HEADS = 12
D_K = 64
N_CORES = 8
H_LOCAL = 3  # heads per core
D_LOCAL = H_LOCAL * D_K  # 192
B = 2
N_TOKENS = 4096
NQ = 512  # query-chunk size (one PSUM bank of fp32)
NT = 512  # token chunk for q/k projection

F32 = mybir.dt.float32
BF16 = mybir.dt.bfloat16
F32R = mybir.dt.float32r


def _mm(ap, flavor):
    """View an fp32 AP as the matmul input dtype."""
    if flavor == "f32r":
        return ap.bitcast(F32R)
    return ap


# Schraudolph fast-exp constants: bitcast(int32(A*x + B)) ~= exp(x) with
# ~3.9% max relative error; the shared-denominator softmax cancels most of
# it. Used to offload part of the exp load from the saturated Scalar
# engine to the Vector engine in the late (exp-dense) chunks.
FEXP_A = float(2 ** 23 / np.log(2))
FEXP_B = float(127.0 * 2 ** 23) - 486408.0


def build_nc(n=N_TOKENS, mm="bf16", dt_x=BF16, dt_pt=BF16, dt_acc=BF16,
             fexp_min_j=10 ** 9):
    # fexp_min_j: first query chunk whose h2 exps run on the Vector engine
    # via the Schraudolph trick instead of the Scalar engine. Measured
    # SLOWER on hardware (392us vs 343us at j>=6 — the Vector engine has
    # less slack in the late chunks than the cost model suggested), so
    # disabled by default; kept for future rebalancing experiments.
    assert n % NQ == 0 and n % NT == 0 and n % P == 0
    nc = bacc.Bacc("TRN2", target_bir_lowering=False, debug=False,
                   num_devices=N_CORES)

    qt_d = nc.dram_tensor("qt", [D_MODEL, n], dt_x, kind="ExternalInput")
    kt_d = nc.dram_tensor("kt", [D_MODEL, n], dt_x, kind="ExternalInput")
    vt_d = nc.dram_tensor("vt", [D_MODEL, n], dt_x, kind="ExternalInput")
    # weights packed host-side into two tensors so the prologue issues two
    # DMA triggers instead of five (each trigger costs ~900ns serially on
    # the Sync queue, which gates time-to-first-matmul)
    wqkv_d = nc.dram_tensor("wqkv", [D_MODEL, 3 * D_LOCAL], dt_x,
                            kind="ExternalInput")
    wocm_d = nc.dram_tensor("wocm", [P, 2 * D_MODEL + P], dt_x,
                            kind="ExternalInput")
    y_d = nc.dram_tensor("y", [n, D_MODEL], dt_acc, kind="ExternalOutput")

    qt_r = qt_d.ap().rearrange("(ko ki) t -> ki ko t", ki=P)
    kt_r = kt_d.ap().rearrange("(ko ki) t -> ki ko t", ki=P)
    vt_r = vt_d.ap().rearrange("(ko ki) t -> ki ko t", ki=P)
    wqkv_r = wqkv_d.ap().rearrange("(ko ki) m -> ki ko m", ki=P)

    TCH = n // NT       # q/k projection token chunks
    TB = n // P         # 128-token blocks
    QCH = n // NQ       # query chunks
    KB_PER_Q = NQ // P  # key blocks per query chunk (4)
    NOC = 2             # output-projection column chunks
    NO = D_MODEL // NOC

    # Host weight-column order: [q0 q1 | q2 k2 | k0 k1] -> 3 full M-blocks.
    # qkT_sb blk3[0:64] is a DMA-shifted copy of k2 (blk1[64:128]) and
    # blk3[64:128] of q2 (blk1[0:64]).
    q_loc = {0: (0, 0), 1: (64, 0), 2: (0, 1)}
    k_loc = {0: (0, 2), 1: (64, 2), 2: (0, 3)}

    with tile.TileContext(nc) as tc:
        with tc.tile_pool(name="const", bufs=1) as cpool, \
             tc.tile_pool(name="persist", bufs=1) as ppool, \
             tc.tile_pool(name="xqk", bufs=4) as xpool, \
             tc.tile_pool(name="xv", bufs=4) as xvpool, \
             tc.tile_pool(name="pt", bufs=12) as ptpool, \
             tc.tile_pool(name="pti", bufs=2) as ptipool, \
             tc.tile_pool(name="ysb", bufs=4) as ypool, \
             tc.tile_pool(name="raw", bufs=4) as rawpool, \
             tc.tile_pool(name="rcp", bufs=6) as rcppool, \
             tc.tile_pool(name="rr", bufs=4) as rrpool, \
             tc.tile_pool(name="ot", bufs=3) as otpool, \
             tc.tile_pool(name="pp_proj", bufs=1, space="PSUM") as pp_proj, \
             tc.tile_pool(name="pp_sc", bufs=2, space="PSUM") as pp_sc, \
             tc.tile_pool(name="pp_out", bufs=1, space="PSUM") as pp_out:

            # ---- constants (two packed DMAs) ----
            wqkv_sb = cpool.tile([P, KO, 3 * D_LOCAL], dt_x)
            nc.sync.dma_start(wqkv_sb[:], wqkv_r)
            wqk_sb = wqkv_sb[:, :, 0:2 * D_LOCAL]
            wv_sb = wqkv_sb[:, :, 2 * D_LOCAL:3 * D_LOCAL]
            # w_o rows (chunk0 = dims of h0,h1; chunk1 = h2 at partitions
            # 0-63) and the causal mask, packed by the host
            wocm_sb = cpool.tile([P, 2 * D_MODEL + P], dt_x)
            nc.sync.dma_start(wocm_sb[:], wocm_d.ap())
            wo_sb = wocm_sb[:, 0:2 * D_MODEL].rearrange("p (c m) -> p c m",
                                                        c=2)
            cm_sb = wocm_sb[:, 2 * D_MODEL:]

            # ---- persistent activations ----
            qkT_sb = ppool.tile([P, 4, n], dt_acc)
            v_sb = ppool.tile([P, TB, H_LOCAL, 66], dt_acc)
            outT_sb = ppool.tile([P, 2, n], dt_acc)
            nc.vector.memset(v_sb[:, :, :, 64:65], 1.0)

            # ---- emission helpers (phases interleaved below) ----

            def emit_qkproj_load(t):
                """Start the q/k input DMAs for chunk t."""
                tok = t * NT
                xq = xpool.tile([P, KO, NT], dt_x, tag="x")
                nc.sync.dma_start(xq[:], qt_r[:, :, tok:tok + NT])
                xk = xpool.tile([P, KO, NT], dt_x, tag="x")
                nc.sync.dma_start(xk[:], kt_r[:, :, tok:tok + NT])
                return xq, xk

            def emit_qkproj_blk(t, blk, xq, xk):
                """Project one 128-row M-block of q/k for chunk t."""
                tok = t * NT
                ps = pp_proj.tile([P, NQ], F32, tag="psproj")
                for ko in range(KO):
                    # blk1 contracts q2 against Q-input and k2 against
                    # K-input: split into two half-partition matmuls.
                    if blk == 1:
                        nc.tensor.matmul(
                            ps[0:64, 0:NT],
                            _mm(wqk_sb[:, ko, 128:192], mm),
                            _mm(xq[:, ko, :], mm),
                            start=(ko == 0), stop=(ko == KO - 1),
                            skip_group_check=True,
                        )
                        nc.tensor.matmul(
                            ps[64:128, 0:NT],
                            _mm(wqk_sb[:, ko, 192:256], mm),
                            _mm(xk[:, ko, :], mm),
                            start=(ko == 0), stop=(ko == KO - 1),
                            skip_group_check=True,
                        )
                    else:
                        x = xq if blk == 0 else xk
                        nc.tensor.matmul(
                            ps[:, 0:NT],
                            _mm(wqk_sb[:, ko, blk * 128:(blk + 1) * 128],
                                mm),
                            _mm(x[:, ko, :], mm),
                            start=(ko == 0), stop=(ko == KO - 1),
                        )
                nc.vector.tensor_copy(
                    out=qkT_sb[:, blk, tok:tok + NT],
                    in_=ps[:, 0:NT],
                )

            def emit_qkproj_shift(t):
                # Partition-shifted copies so h2's scores matmul sees qT/kT
                # at the same base — and at BOTH bases, so h2 can alternate
                # row-groups and pair with whichever half is free:
                #   blk3[0:64]   = k2 (from blk1[64:128])
                #   blk3[64:128] = q2 (from blk1[0:64])
                tok = t * NT
                nc.sync.dma_start(
                    qkT_sb[0:64, 3, tok:tok + NT],
                    qkT_sb[64:128, 1, tok:tok + NT],
                )
                nc.sync.dma_start(
                    qkT_sb[64:128, 3, tok:tok + NT],
                    qkT_sb[0:64, 1, tok:tok + NT],
                )

            def emit_qkproj(t):
                """Project q/k for tokens [t*NT, (t+1)*NT) into qkT_sb."""
                xq, xk = emit_qkproj_load(t)
                for blk in range(3):
                    emit_qkproj_blk(t, blk, xq, xk)
                emit_qkproj_shift(t)

            def emit_vproj_group(tb0, ntb):
                """Project v for token blocks [tb0, tb0+ntb) with a single
                input DMA (DMA triggers serialize on the Sync queue)."""
                xv = xvpool.tile([P, KO, KB_PER_Q * P], dt_x)
                nc.sync.dma_start(xv[:, :, 0:ntb * P],
                                  vt_r[:, :, tb0 * P:(tb0 + ntb) * P])
                for i in range(ntb):
                    ps = pp_proj.tile([P, NQ], F32, tag="psproj")
                    for ko in range(KO):
                        nc.tensor.matmul(
                            ps[:, 0:D_LOCAL],
                            _mm(xv[:, ko, i * P:(i + 1) * P], mm),
                            _mm(wv_sb[:, ko, :], mm),
                            start=(ko == 0), stop=(ko == KO - 1),
                        )
                    # free sizes match (192); AP element order is h-major
                    # on both sides so a single strided copy works
                    nc.vector.tensor_copy(
                        out=v_sb[:, tb0 + i, :, 0:64],
                        in_=ps[:, 0:D_LOCAL],
                    )

            def emit_outproj(tb, tail=False):
                """y[tb*128:(tb+1)*128, :] = outT[:, tb-block].T @ w_o.

                In the epilogue the attention PSUM banks are free, so use
                the (2-bank) score pool for double-buffering instead of
                serializing on the single projection bank."""
                for oc in range(NOC):
                    if tail:
                        ps = pp_sc.tile([P, 2, NQ], F32, tag="psc",
                                        name="psc")[:, 0, :]
                    else:
                        ps = pp_proj.tile([P, NQ], F32, tag="psproj")
                    nc.tensor.matmul(
                        ps[:, 0:NO],
                        _mm(outT_sb[:, 0, tb * P:(tb + 1) * P], mm),
                        _mm(wo_sb[:, 0, oc * NO:(oc + 1) * NO], mm),
                        start=True, stop=False,
                    )
                    nc.tensor.matmul(
                        ps[:, 0:NO],
                        _mm(outT_sb[0:64, 1, tb * P:(tb + 1) * P], mm),
                        _mm(wo_sb[0:64, 1, oc * NO:(oc + 1) * NO], mm),
                        start=False, stop=True,
                    )
                    ysb = ypool.tile([P, NO], dt_acc)
                    nc.vector.tensor_copy(out=ysb[:], in_=ps[:, 0:NO])
                    nc.sync.dma_start(
                        y_d.ap()[tb * P:(tb + 1) * P, oc * NO:(oc + 1) * NO],
                        ysb[:],
                    )

            # Heads are interleaved so the PE runs two concurrent score
            # matmuls on disjoint row-groups: h0 lives at partitions 0-63,
            # h1 at 64-127, h2 alternates base per key-block (its qT/kT are
            # replicated at both bases in blk1/blk3).
            def h2_qk(kb):
                if kb % 2 == 0:
                    return (0, 1), (0, 3)   # q2 @ blk1[0:64], k2' @ blk3[0:64]
                return (64, 3), (64, 1)     # q2' @ blk3[64:128], k2 @ blk1[64:128]

            def qk_for(h, kb):
                if h == 2:
                    return h2_qk(kb)
                return q_loc[h], k_loc[h]

            def emit_scores_pair(j, kb2):
                """Scores + exp + mask for key blocks kb2, kb2+1."""
                offs = {}
                pt2s = {}
                for h in range(H_LOCAL):
                    psc2 = pp_sc.tile([P, 2, NQ], F32, tag="psc", name="psc")
                    # Both blocks' matmuls start at the PAIR's minimum
                    # offset so the wide ACT below reads no uninitialized
                    # PSUM; the extra columns of the right (diagonal) block
                    # are never read by its AV matmul.
                    off0 = max(kb2 - KB_PER_Q * j, 0) * P
                    for i, kb in ((0, kb2), (1, kb2 + 1)):
                        (qp, qb), (kp, kb_) = qk_for(h, kb)
                        kloc = kb - KB_PER_Q * j
                        offs[(h, i)] = max(kloc, 0) * P  # mask/AV offset
                        nc.tensor.matmul(
                            psc2[:, i, off0:],
                            _mm(qkT_sb[kp:kp + 64, kb_, kb * P:(kb + 1) * P],
                                mm),
                            _mm(qkT_sb[qp:qp + 64, qb,
                                       j * NQ + off0:(j + 1) * NQ], mm),
                            start=True, stop=True,
                        )
                    pt2 = ptpool.tile([P, 2, NQ], dt_pt, name="pt")
                    if h == 2 and j >= fexp_min_j and dt_pt == BF16:
                        # Offload h2's exp to the Vector engine via the
                        # Schraudolph int trick in the late chunks where
                        # the Scalar engine is saturated: int32(A*x+B)'s
                        # bit pattern is ~exp(x); its high 16 bits ARE the
                        # bf16 of that value (truncated mantissa).
                        pti = ptipool.tile([P, 2, NQ], mybir.dt.int32,
                                           name="pti")
                        nc.vector.tensor_scalar(
                            out=pti[:, :, off0:], in0=psc2[:, :, off0:],
                            scalar1=FEXP_A, scalar2=FEXP_B,
                            op0=mybir.AluOpType.mult,
                            op1=mybir.AluOpType.add)
                        hi = pti.bitcast(mybir.dt.uint16).rearrange(
                            "p a (w t) -> p a w t", t=2)
                        nc.vector.tensor_copy(
                            out=pt2.bitcast(mybir.dt.uint16)[:, :, off0:],
                            in_=hi[:, :, off0:, 1:2])
                    else:
                        # One wide ACT over both key blocks (2 PSUM banks).
                        nc.scalar.activation(pt2[:, :, off0:],
                                             psc2[:, :, off0:],
                                             mybir.ActivationFunctionType.Exp)
                    for i, kb in ((0, kb2), (1, kb2 + 1)):
                        kloc = kb - KB_PER_Q * j
                        if kloc >= 0:
                            off = offs[(h, i)]
                            nc.vector.tensor_mul(out=pt2[:, i, off:off + P],
                                                 in0=pt2[:, i, off:off + P],
                                                 in1=cm_sb[:])
                    pt2s[h] = pt2
                return pt2s, offs

            def emit_av_pair(j, po, kb2, nkb, pt2s, offs):
                """AV accumulation for key blocks kb2, kb2+1 (emitted one
                pair behind the scores so the Scalar engine always has the
                next pair's scores queued ahead of AV work on the PE)."""
                for h in range(H_LOCAL):
                    pt2 = pt2s[h]
                    for i, kb in ((0, kb2), (1, kb2 + 1)):
                        off = offs[(h, i)]
                        nc.tensor.matmul(
                            po[h][0:65, off:],
                            _mm(v_sb[:, kb, h, 0:65], mm),
                            _mm(pt2[:, i, off:], mm),
                            start=(kb == 0), stop=(kb == nkb - 1),
                        )

            def emit_normalize(j, po, c0=0, c1=NQ):
                """Divide outT_aug rows by the denominator row for chunk
                j's query columns [c0, c1) (the low half of a chunk is
                final two AV pairs before the high half — splitting lets
                the epilogue start earlier)."""
                w = c1 - c0
                for h in range(H_LOCAL):
                    raw = rawpool.tile([65, NQ], F32, tag="raw", name="raw")
                    nc.vector.tensor_copy(out=raw[:, 0:w],
                                          in_=po[h][0:65, c0:c1])
                    # DVE lanes and the gpsimd broadcast's reader are
                    # partition-locked: move the denominator row (SBUF
                    # partition 64) to partition 0 with a DMA first.
                    den0 = rcppool.tile([1, NQ], F32, tag="den0", name="den0")
                    nc.sync.dma_start(den0[:, 0:w], raw[64:65, 0:w])
                    rcp = rcppool.tile([1, NQ], F32, tag="rcp", name="rcp")
                    nc.vector.reciprocal_approx_fast(out=rcp[:, 0:w],
                                                     in_=den0[:, 0:w])
                    rr = rrpool.tile([64, NQ], F32, tag="rr", name="rr")
                    nc.gpsimd.partition_broadcast(rr[:, 0:w], rcp[:, 0:w],
                                                  channels=64)
                    q0 = j * NQ + c0
                    if h == 1:
                        # h1 lives at partitions 64-127 of outT blk0; DVE
                        # lanes are partition-locked, so write a temp at
                        # base 0 and DMA partition-shift it up.
                        ot = otpool.tile([64, NQ], dt_acc, name="ot")
                        nc.vector.tensor_mul(out=ot[:, 0:w],
                                             in0=raw[0:64, 0:w],
                                             in1=rr[:, 0:w])
                        nc.sync.dma_start(
                            outT_sb[64:128, 0, q0:q0 + w], ot[:, 0:w])
                    else:
                        dst = outT_sb[0:64, 0 if h == 0 else 1, q0:q0 + w]
                        nc.vector.tensor_mul(out=dst, in0=raw[0:64, 0:w],
                                             in1=rr[:, 0:w])

            # ---- software-pipelined emission ----
            # Prologue: everything attention chunk 0 needs.
            emit_qkproj(0)
            emit_vproj_group(0, min(KB_PER_Q, TB))

            # Output projections are deferred to the late chunks, where the
            # Scalar engine is saturated with exps and the PE has slack; in
            # the early chunks the PE is the bottleneck.
            op_start = max(QCH - 3, 1)
            next_tb = 0

            for j in range(QCH):
                po = [pp_out.tile([P, NQ], F32, tag=f"po{h}", name=f"po{h}")
                      for h in range(H_LOCAL)]
                nkb = KB_PER_Q * (j + 1)
                npairs = nkb // 2
                pend = []  # scores emitted, AV not yet (2 pairs deep)
                for p in range(npairs):
                    pend.append((2 * p, emit_scores_pair(j, 2 * p)))
                    if len(pend) > 2:
                        kb2p, args = pend.pop(0)
                        emit_av_pair(j, po, kb2p, nkb, *args)
                    # Interleave next-chunk projections and the previous
                    # chunk's output projection between attention groups so
                    # the PE stays busy while the Scalar engine (the
                    # bottleneck) churns through the wide exps.
                    if p == 0 and j + 1 < TCH:
                        emit_qkproj(j + 1)
                    elif p == 1 or (npairs == 1 and p == 0):
                        tb0 = KB_PER_Q * (j + 1)
                        ntb = min(KB_PER_Q * (j + 2), TB) - tb0
                        if ntb > 0:
                            emit_vproj_group(tb0, ntb)
                    elif (j >= op_start and p >= 2 and p % 2 == 0
                          and p <= npairs - 2):
                        for _ in range(4):
                            if next_tb < KB_PER_Q * j:
                                emit_outproj(next_tb)
                                next_tb += 1
                for kb2p, args in pend:
                    emit_av_pair(j, po, kb2p, nkb, *args)
                emit_normalize(j, po)

            # Epilogue: whatever output projections remain (at least the
            # final query chunk, whose outT rows only exist after the last
            # normalize). The attention PSUM banks are free here, so
            # double-buffer from the score pool instead of serializing on
            # the single projection bank.
            for tb in range(next_tb, TB):
                emit_outproj(tb, tail=True)

    nc.compile()
    return nc


def make_causal_mask_np(dt=np.float32):
    """[128, 128] lower-left keep mask: m[p, f] = 1.0 iff f >= p."""
    f = np.arange(P)[None, :]
    p = np.arange(P)[:, None]
    return (f >= p).astype(np.float32).astype(dt)


def prep_core_inputs(Q, K, V, w_q, w_k, w_v, w_o, core, n=N_TOKENS,
                     np_x=ml_dtypes.bfloat16, np_pt=ml_dtypes.bfloat16):
    """Host-side sharding/layout prep for one core. All fp32 numpy in."""
    b = core // 4
    g = core % 4
    hs = g * D_LOCAL
    scale = 1.0 / np.sqrt(D_K)
    qt = np.ascontiguousarray(Q[b].T).astype(np_x)
    kt = np.ascontiguousarray(K[b].T).astype(np_x)
    vt = np.ascontiguousarray(V[b].T).astype(np_x)
    wql = w_q[hs:hs + D_LOCAL] * scale
    wkl = w_k[hs:hs + D_LOCAL]
    # column order [q0 q1 | q2 k2 | k0 k1 | v] (see build_nc); q/k/v
    # packed into one tensor so the device loads them with one DMA
    wqkv = np.ascontiguousarray(
        np.concatenate([wql[0:128], wql[128:192], wkl[128:192], wkl[0:128],
                        w_v[hs:hs + D_LOCAL]], axis=0).T
    ).astype(np_x)
    # w_o row-chunks ([0:128] then [128:192] at partitions 0-63) and the
    # causal mask, packed into one [128, 2*768+128] tensor
    wo = w_o[:, hs:hs + D_LOCAL].T
    wocm = np.zeros((P, 2 * D_MODEL + P), dtype=np.float32)
    wocm[:, 0:D_MODEL] = wo[0:P]
    wocm[0:64, D_MODEL:2 * D_MODEL] = wo[P:D_LOCAL]
    wocm[:, 2 * D_MODEL:] = make_causal_mask_np(np.float32)
    return {"qt": qt, "kt": kt, "vt": vt,
            "wqkv": wqkv, "wocm": wocm.astype(np_x)}


_NC_CACHE = {}


def _get_nc(key, **kw):
    if key not in _NC_CACHE:
        _NC_CACHE[key] = build_nc(**kw)
    return _NC_CACHE[key]


KCFG = {"mm": "bf16", "dt_x": BF16, "dt_pt": BF16, "dt_acc": BF16,
        "np_x": ml_dtypes.bfloat16, "np_pt": ml_dtypes.bfloat16}


def kernel(Q, K, V, w_q, w_k, w_v, w_o):
    Q = np.asarray(Q, dtype=np.float32)
    K = np.asarray(K, dtype=np.float32)
    V = np.asarray(V, dtype=np.float32)
    w_q = np.asarray(w_q, dtype=np.float32)
    w_k = np.asarray(w_k, dtype=np.float32)
    w_v = np.asarray(w_v, dtype=np.float32)
    w_o = np.asarray(w_o, dtype=np.float32)

    nc = _get_nc((KCFG["mm"], str(KCFG["dt_x"])),
                 n=N_TOKENS, mm=KCFG["mm"], dt_x=KCFG["dt_x"],
                 dt_pt=KCFG["dt_pt"], dt_acc=KCFG["dt_acc"])
    in_maps = [
        prep_core_inputs(Q, K, V, w_q, w_k, w_v, w_o, c,
                         np_x=KCFG["np_x"], np_pt=KCFG["np_pt"])
        for c in range(N_CORES)
    ]
    res = bass_utils.run_bass_kernel_spmd(nc, in_maps,
                                          core_ids=list(range(N_CORES)))
    out = np.zeros((B, N_TOKENS, D_MODEL), dtype=np.float32)
    for c in range(N_CORES):
        out[c // 4] += np.asarray(res.results[c]["y"], dtype=np.float32)
    return out


# revision 59
# speedup vs baseline: 1.7475x; 1.0059x over previous
"""Trainium2 Bass kernel for causal MHA (b=2, n=4096, d_model=768, 12 heads).

Sharding: 8 cores = 2 batches x 4 head-groups (3 heads each).
Each core:
  - receives its batch's Q/K/V pre-transposed ([768, n], d_model on rows)
    plus its head-group's weight slices (also pre-transposed on host).
  - projects qT/kT ([64, n] per head, head dim on partitions) and
    v ([n, 64] per head, tokens on partitions) on-chip.
  - computes scoresT[k, q] = kT^T @ qT for key-block PAIRS into a 2-bank
    PSUM tile, exponentiates both blocks with ONE wide ACT instruction
    (halves the per-instruction overhead on the bottleneck engine),
    masks the causal boundary blocks, and accumulates
    outT_aug[65, q] += [v | ones]^T @ P in PSUM (row 64 = denominator).
  - normalizes via reciprocal_approx_fast + gpsimd partition_broadcast
    (no DRAM bounce, no 8-cycle/elem reciprocal).
  - applies the output projection with its w_o row-slice; host sums the
    4 bf16 partial outputs per batch (row-parallel linear unshard).

The emission order software-pipelines the phases: q/k/v projection
chunks and the output projection of the previous query chunk are
interleaved between attention pair-groups, so the Scalar engine (the
bottleneck: ~26M exp elements) is kept fed from the first microsecond
and the PE never idles long enough to lose its HAM clock boost.

Weight-column host layout packs the six 64-wide q/k heads into three full
128-row M-blocks ([q0;q1], [q2;k2], [k0;k1]); k2/q2 are then DMA-copied to
a fourth block so every head's scores matmul sees its qT and kT at the
same partition base (a matmul constraint), with h2 replicated at both
bases so it can alternate row-groups per key block and pair with itself.
"""

import sys

for _p in ("/opt/trn_rl_repo",):
    if _p not in sys.path:
        sys.path.insert(0, _p)

import numpy as np
import ml_dtypes

import concourse.bass as bass  # noqa: F401  (registers engine classes)
import concourse.tile as tile
from concourse import bacc, mybir
import concourse.bass_utils as bass_utils

P = 128
D_MODEL = 768
KO = D_MODEL // P  # 6 contraction chunks of 128
N_HEADS = 12
D_K = 64
N_CORES = 8
H_LOCAL = 3  # heads per core
D_LOCAL = H_LOCAL * D_K  # 192
B = 2
N_TOKENS = 4096
NQ = 512  # query-chunk size (one PSUM bank of fp32)
NT = 512  # token chunk for q/k projection

F32 = mybir.dt.float32
BF16 = mybir.dt.bfloat16
F32R = mybir.dt.float32r


def _mm(ap, flavor):
    """View an fp32 AP as the matmul input dtype."""
    if flavor == "f32r":
        return ap.bitcast(F32R)
    return ap


# Schraudolph fast-exp constants: bitcast(int32(A*x + B)) ~= exp(x) with
# ~3.9% max relative error; the shared-denominator softmax cancels most of
# it. Used to offload part of the exp load from the saturated Scalar
# engine to the Vector engine in the late (exp-dense) chunks.
FEXP_A = float(2 ** 23 / np.log(2))
FEXP_B = float(127.0 * 2 ** 23) - 486408.0


def build_nc(n=N_TOKENS, mm="bf16", dt_x=BF16, dt_pt=BF16, dt_acc=BF16,
             fexp_min_j=10 ** 9):
    # fexp_min_j: first query chunk whose h2 exps run on the Vector engine
    # via the Schraudolph trick instead of the Scalar engine. Measured
    # SLOWER on hardware (392us vs 343us at j>=6 — the Vector engine has
    # less slack in the late chunks than the cost model suggested), so
    # disabled by default; kept for future rebalancing experiments.
    assert n % NQ == 0 and n % NT == 0 and n % P == 0
    nc = bacc.Bacc("TRN2", target_bir_lowering=False, debug=False,
                   num_devices=N_CORES)

    qt_d = nc.dram_tensor("qt", [D_MODEL, n], dt_x, kind="ExternalInput")
    kt_d = nc.dram_tensor("kt", [D_MODEL, n], dt_x, kind="ExternalInput")
    vt_d = nc.dram_tensor("vt", [D_MODEL, n], dt_x, kind="ExternalInput")
    # weights packed host-side into two tensors so the prologue issues two
    # DMA triggers instead of five (each trigger costs ~900ns serially on
    # the Sync queue, which gates time-to-first-matmul)
    wqkv_d = nc.dram_tensor("wqkv", [D_MODEL, 3 * D_LOCAL], dt_x,
                            kind="ExternalInput")
    wocm_d = nc.dram_tensor("wocm", [P, 2 * D_MODEL + P], dt_x,
                            kind="ExternalInput")
    y_d = nc.dram_tensor("y", [n, D_MODEL], dt_acc, kind="ExternalOutput")

    qt_r = qt_d.ap().rearrange("(ko ki) t -> ki ko t", ki=P)
    kt_r = kt_d.ap().rearrange("(ko ki) t -> ki ko t", ki=P)
    vt_r = vt_d.ap().rearrange("(ko ki) t -> ki ko t", ki=P)
    wqkv_r = wqkv_d.ap().rearrange("(ko ki) m -> ki ko m", ki=P)

    TCH = n // NT       # q/k projection token chunks
    TB = n // P         # 128-token blocks
    QCH = n // NQ       # query chunks
    KB_PER_Q = NQ // P  # key blocks per query chunk (4)
    NOC = 2             # output-projection column chunks
    NO = D_MODEL // NOC

    # Host weight-column order: [q0 q1 | q2 k2 | k0 k1] -> 3 full M-blocks.
    # qkT_sb blk3[0:64] is a DMA-shifted copy of k2 (blk1[64:128]) and
    # blk3[64:128] of q2 (blk1[0:64]).
    q_loc = {0: (0, 0), 1: (64, 0), 2: (0, 1)}
    k_loc = {0: (0, 2), 1: (64, 2), 2: (0, 3)}

    with tile.TileContext(nc) as tc:
        with tc.tile_pool(name="const", bufs=1) as cpool, \
             tc.tile_pool(name="persist", bufs=1) as ppool, \
             tc.tile_pool(name="xqk", bufs=6) as xpool, \
             tc.tile_pool(name="xv", bufs=4) as xvpool, \
             tc.tile_pool(name="pt", bufs=12) as ptpool, \
             tc.tile_pool(name="pti", bufs=2) as ptipool, \
             tc.tile_pool(name="ysb", bufs=4) as ypool, \
             tc.tile_pool(name="raw", bufs=4) as rawpool, \
             tc.tile_pool(name="rcp", bufs=6) as rcppool, \
             tc.tile_pool(name="rr", bufs=4) as rrpool, \
             tc.tile_pool(name="ot", bufs=3) as otpool, \
             tc.tile_pool(name="pp_proj", bufs=1, space="PSUM") as pp_proj, \
             tc.tile_pool(name="pp_sc", bufs=2, space="PSUM") as pp_sc, \
             tc.tile_pool(name="pp_out", bufs=1, space="PSUM") as pp_out:

            # ---- constants (two packed DMAs) ----
            wqkv_sb = cpool.tile([P, KO, 3 * D_LOCAL], dt_x)
            nc.sync.dma_start(wqkv_sb[:], wqkv_r)
            wqk_sb = wqkv_sb[:, :, 0:2 * D_LOCAL]
            wv_sb = wqkv_sb[:, :, 2 * D_LOCAL:3 * D_LOCAL]
            # w_o rows (chunk0 = dims of h0,h1; chunk1 = h2 at partitions
            # 0-63) and the causal mask, packed by the host
            wocm_sb = cpool.tile([P, 2 * D_MODEL + P], dt_x)
            nc.sync.dma_start(wocm_sb[:], wocm_d.ap())
            wo_sb = wocm_sb[:, 0:2 * D_MODEL].rearrange("p (c m) -> p c m",
                                                        c=2)
            cm_sb = wocm_sb[:, 2 * D_MODEL:]

            # ---- persistent activations ----
            qkT_sb = ppool.tile([P, 4, n], dt_acc)
            v_sb = ppool.tile([P, TB, H_LOCAL, 66], dt_acc)
            outT_sb = ppool.tile([P, 2, n], dt_acc)
            nc.vector.memset(v_sb[:, :, :, 64:65], 1.0)

            # ---- emission helpers (phases interleaved below) ----

            def emit_qkproj_load(t):
                """Start the q/k input DMAs for chunk t."""
                tok = t * NT
                xq = xpool.tile([P, KO, NT], dt_x, tag="x")
                nc.sync.dma_start(xq[:], qt_r[:, :, tok:tok + NT])
                xk = xpool.tile([P, KO, NT], dt_x, tag="x")
                nc.sync.dma_start(xk[:], kt_r[:, :, tok:tok + NT])
                return xq, xk

            def emit_qkproj_blk(t, blk, xq, xk):
                """Project one 128-row M-block of q/k for chunk t."""
                tok = t * NT
                ps = pp_proj.tile([P, NQ], F32, tag="psproj")
                for ko in range(KO):
                    # blk1 contracts q2 against Q-input and k2 against
                    # K-input: split into two half-partition matmuls.
                    if blk == 1:
                        nc.tensor.matmul(
                            ps[0:64, 0:NT],
                            _mm(wqk_sb[:, ko, 128:192], mm),
                            _mm(xq[:, ko, :], mm),
                            start=(ko == 0), stop=(ko == KO - 1),
                            skip_group_check=True,
                        )
                        nc.tensor.matmul(
                            ps[64:128, 0:NT],
                            _mm(wqk_sb[:, ko, 192:256], mm),
                            _mm(xk[:, ko, :], mm),
                            start=(ko == 0), stop=(ko == KO - 1),
                            skip_group_check=True,
                        )
                    else:
                        x = xq if blk == 0 else xk
                        nc.tensor.matmul(
                            ps[:, 0:NT],
                            _mm(wqk_sb[:, ko, blk * 128:(blk + 1) * 128],
                                mm),
                            _mm(x[:, ko, :], mm),
                            start=(ko == 0), stop=(ko == KO - 1),
                        )
                nc.vector.tensor_copy(
                    out=qkT_sb[:, blk, tok:tok + NT],
                    in_=ps[:, 0:NT],
                )

            def emit_qkproj_shift(t):
                # Partition-shifted copies so h2's scores matmul sees qT/kT
                # at the same base — and at BOTH bases, so h2 can alternate
                # row-groups and pair with whichever half is free:
                #   blk3[0:64]   = k2 (from blk1[64:128])
                #   blk3[64:128] = q2 (from blk1[0:64])
                tok = t * NT
                nc.sync.dma_start(
                    qkT_sb[0:64, 3, tok:tok + NT],
                    qkT_sb[64:128, 1, tok:tok + NT],
                )
                nc.sync.dma_start(
                    qkT_sb[64:128, 3, tok:tok + NT],
                    qkT_sb[0:64, 1, tok:tok + NT],
                )

            def emit_qkproj(t):
                """Project q/k for tokens [t*NT, (t+1)*NT) into qkT_sb."""
                xq, xk = emit_qkproj_load(t)
                for blk in range(3):
                    emit_qkproj_blk(t, blk, xq, xk)
                emit_qkproj_shift(t)

            def emit_vproj_group(tb0, ntb):
                """Project v for token blocks [tb0, tb0+ntb) with a single
                input DMA (DMA triggers serialize on the Sync queue)."""
                xv = xvpool.tile([P, KO, KB_PER_Q * P], dt_x)
                nc.sync.dma_start(xv[:, :, 0:ntb * P],
                                  vt_r[:, :, tb0 * P:(tb0 + ntb) * P])
                for i in range(ntb):
                    ps = pp_proj.tile([P, NQ], F32, tag="psproj")
                    for ko in range(KO):
                        nc.tensor.matmul(
                            ps[:, 0:D_LOCAL],
                            _mm(xv[:, ko, i * P:(i + 1) * P], mm),
                            _mm(wv_sb[:, ko, :], mm),
                            start=(ko == 0), stop=(ko == KO - 1),
                        )
                    # free sizes match (192); AP element order is h-major
                    # on both sides so a single strided copy works
                    nc.vector.tensor_copy(
                        out=v_sb[:, tb0 + i, :, 0:64],
                        in_=ps[:, 0:D_LOCAL],
                    )

            def emit_outproj(tb, tail=False):
                """y[tb*128:(tb+1)*128, :] = outT[:, tb-block].T @ w_o.

                In the epilogue the attention PSUM banks are free, so use
                the (2-bank) score pool for double-buffering instead of
                serializing on the single projection bank."""
                for oc in range(NOC):
                    if tail:
                        ps = pp_sc.tile([P, 2, NQ], F32, tag="psc",
                                        name="psc")[:, 0, :]
                    else:
                        ps = pp_proj.tile([P, NQ], F32, tag="psproj")
                    nc.tensor.matmul(
                        ps[:, 0:NO],
                        _mm(outT_sb[:, 0, tb * P:(tb + 1) * P], mm),
                        _mm(wo_sb[:, 0, oc * NO:(oc + 1) * NO], mm),
                        start=True, stop=False,
                    )
                    nc.tensor.matmul(
                        ps[:, 0:NO],
                        _mm(outT_sb[0:64, 1, tb * P:(tb + 1) * P], mm),
                        _mm(wo_sb[0:64, 1, oc * NO:(oc + 1) * NO], mm),
                        start=False, stop=True,
                    )
                    ysb = ypool.tile([P, NO], dt_acc)
                    nc.vector.tensor_copy(out=ysb[:], in_=ps[:, 0:NO])
                    nc.sync.dma_start(
                        y_d.ap()[tb * P:(tb + 1) * P, oc * NO:(oc + 1) * NO],
                        ysb[:],
                    )

            # Heads are interleaved so the PE runs two concurrent score
            # matmuls on disjoint row-groups: h0 lives at partitions 0-63,
            # h1 at 64-127, h2 alternates base per key-block (its qT/kT are
            # replicated at both bases in blk1/blk3).
            def h2_qk(kb):
                if kb % 2 == 0:
                    return (0, 1), (0, 3)   # q2 @ blk1[0:64], k2' @ blk3[0:64]
                return (64, 3), (64, 1)     # q2' @ blk3[64:128], k2 @ blk1[64:128]

            def qk_for(h, kb):
                if h == 2:
                    return h2_qk(kb)
                return q_loc[h], k_loc[h]

            def emit_scores_pair(j, kb2):
                """Scores + exp + mask for key blocks kb2, kb2+1."""
                offs = {}
                pt2s = {}
                for h in range(H_LOCAL):
                    psc2 = pp_sc.tile([P, 2, NQ], F32, tag="psc", name="psc")
                    # Both blocks' matmuls start at the PAIR's minimum
                    # offset so the wide ACT below reads no uninitialized
                    # PSUM; the extra columns of the right (diagonal) block
                    # are never read by its AV matmul.
                    off0 = max(kb2 - KB_PER_Q * j, 0) * P
                    for i, kb in ((0, kb2), (1, kb2 + 1)):
                        (qp, qb), (kp, kb_) = qk_for(h, kb)
                        kloc = kb - KB_PER_Q * j
                        offs[(h, i)] = max(kloc, 0) * P  # mask/AV offset
                        nc.tensor.matmul(
                            psc2[:, i, off0:],
                            _mm(qkT_sb[kp:kp + 64, kb_, kb * P:(kb + 1) * P],
                                mm),
                            _mm(qkT_sb[qp:qp + 64, qb,
                                       j * NQ + off0:(j + 1) * NQ], mm),
                            start=True, stop=True,
                        )
                    pt2 = ptpool.tile([P, 2, NQ], dt_pt, name="pt")
                    if h == 2 and j >= fexp_min_j and dt_pt == BF16:
                        # Offload h2's exp to the Vector engine via the
                        # Schraudolph int trick in the late chunks where
                        # the Scalar engine is saturated: int32(A*x+B)'s
                        # bit pattern is ~exp(x); its high 16 bits ARE the
                        # bf16 of that value (truncated mantissa).
                        pti = ptipool.tile([P, 2, NQ], mybir.dt.int32,
                                           name="pti")
                        nc.vector.tensor_scalar(
                            out=pti[:, :, off0:], in0=psc2[:, :, off0:],
                            scalar1=FEXP_A, scalar2=FEXP_B,
                            op0=mybir.AluOpType.mult,
                            op1=mybir.AluOpType.add)
                        hi = pti.bitcast(mybir.dt.uint16).rearrange(
                            "p a (w t) -> p a w t", t=2)
                        nc.vector.tensor_copy(
                            out=pt2.bitcast(mybir.dt.uint16)[:, :, off0:],
                            in_=hi[:, :, off0:, 1:2])
                    else:
                        # One wide ACT over both key blocks (2 PSUM banks).
                        nc.scalar.activation(pt2[:, :, off0:],
                                             psc2[:, :, off0:],
                                             mybir.ActivationFunctionType.Exp)
                    for i, kb in ((0, kb2), (1, kb2 + 1)):
                        kloc = kb - KB_PER_Q * j
                        if kloc >= 0:
                            off = offs[(h, i)]
                            nc.vector.tensor_mul(out=pt2[:, i, off:off + P],
                                                 in0=pt2[:, i, off:off + P],
                                                 in1=cm_sb[:])
                    pt2s[h] = pt2
                return pt2s, offs

            def emit_av_pair(j, po, kb2, nkb, pt2s, offs):
                """AV accumulation for key blocks kb2, kb2+1 (emitted one
                pair behind the scores so the Scalar engine always has the
                next pair's scores queued ahead of AV work on the PE)."""
                for h in range(H_LOCAL):
                    pt2 = pt2s[h]
                    for i, kb in ((0, kb2), (1, kb2 + 1)):
                        off = offs[(h, i)]
                        nc.tensor.matmul(
                            po[h][0:65, off:],
                            _mm(v_sb[:, kb, h, 0:65], mm),
                            _mm(pt2[:, i, off:], mm),
                            start=(kb == 0), stop=(kb == nkb - 1),
                        )

            def emit_normalize(j, po, c0=0, c1=NQ):
                """Divide outT_aug rows by the denominator row for chunk
                j's query columns [c0, c1) (the low half of a chunk is
                final two AV pairs before the high half — splitting lets
                the epilogue start earlier)."""
                w = c1 - c0
                for h in range(H_LOCAL):
                    raw = rawpool.tile([65, NQ], F32, tag="raw", name="raw")
                    nc.vector.tensor_copy(out=raw[:, 0:w],
                                          in_=po[h][0:65, c0:c1])
                    # DVE lanes and the gpsimd broadcast's reader are
                    # partition-locked: move the denominator row (SBUF
                    # partition 64) to partition 0 with a DMA first.
                    den0 = rcppool.tile([1, NQ], F32, tag="den0", name="den0")
                    nc.sync.dma_start(den0[:, 0:w], raw[64:65, 0:w])
                    rcp = rcppool.tile([1, NQ], F32, tag="rcp", name="rcp")
                    nc.vector.reciprocal_approx_fast(out=rcp[:, 0:w],
                                                     in_=den0[:, 0:w])
                    rr = rrpool.tile([64, NQ], F32, tag="rr", name="rr")
                    nc.gpsimd.partition_broadcast(rr[:, 0:w], rcp[:, 0:w],
                                                  channels=64)
                    q0 = j * NQ + c0
                    if h == 1:
                        # h1 lives at partitions 64-127 of outT blk0; DVE
                        # lanes are partition-locked, so write a temp at
                        # base 0 and DMA partition-shift it up.
                        ot = otpool.tile([64, NQ], dt_acc, name="ot")
                        nc.vector.tensor_mul(out=ot[:, 0:w],
                                             in0=raw[0:64, 0:w],
                                             in1=rr[:, 0:w])
                        nc.sync.dma_start(
                            outT_sb[64:128, 0, q0:q0 + w], ot[:, 0:w])
                    else:
                        dst = outT_sb[0:64, 0 if h == 0 else 1, q0:q0 + w]
                        nc.vector.tensor_mul(out=dst, in0=raw[0:64, 0:w],
                                             in1=rr[:, 0:w])

            # ---- software-pipelined emission ----
            # Prologue: everything attention chunk 0 needs.
            emit_qkproj(0)
            emit_vproj_group(0, min(KB_PER_Q, TB))

            # Output projections are deferred to the late chunks, where the
            # Scalar engine is saturated with exps and the PE has slack; in
            # the early chunks the PE is the bottleneck.
            op_start = max(QCH - 3, 1)
            next_tb = 0

            for j in range(QCH):
                po = [pp_out.tile([P, NQ], F32, tag=f"po{h}", name=f"po{h}")
                      for h in range(H_LOCAL)]
                nkb = KB_PER_Q * (j + 1)
                npairs = nkb // 2
                pend = []  # scores emitted, AV not yet (2 pairs deep)
                for p in range(npairs):
                    pend.append((2 * p, emit_scores_pair(j, 2 * p)))
                    if len(pend) > 2:
                        kb2p, args = pend.pop(0)
                        emit_av_pair(j, po, kb2p, nkb, *args)
                    # Interleave next-chunk projections and the previous
                    # chunk's output projection between attention groups so
                    # the PE stays busy while the Scalar engine (the
                    # bottleneck) churns through the wide exps.
                    if p == 0 and j + 1 < TCH:
                        emit_qkproj(j + 1)
                    elif p == 1 or (npairs == 1 and p == 0):
                        tb0 = KB_PER_Q * (j + 1)
                        ntb = min(KB_PER_Q * (j + 2), TB) - tb0
                        if ntb > 0:
                            emit_vproj_group(tb0, ntb)
                    elif (j >= op_start and p >= 2 and p % 2 == 0
                          and p <= npairs - 2):
                        for _ in range(4):
                            if next_tb < KB_PER_Q * j:
                                emit_outproj(next_tb)
                                next_tb += 1
                for kb2p, args in pend:
                    emit_av_pair(j, po, kb2p, nkb, *args)
                emit_normalize(j, po)

            # Epilogue: whatever output projections remain (at least the
            # final query chunk, whose outT rows only exist after the last
            # normalize). The attention PSUM banks are free here, so
            # double-buffer from the score pool instead of serializing on
            # the single projection bank.
            for tb in range(next_tb, TB):
                emit_outproj(tb, tail=True)

    nc.compile()
    return nc


def make_causal_mask_np(dt=np.float32):
    """[128, 128] lower-left keep mask: m[p, f] = 1.0 iff f >= p."""
    f = np.arange(P)[None, :]
    p = np.arange(P)[:, None]
    return (f >= p).astype(np.float32).astype(dt)


def prep_core_inputs(Q, K, V, w_q, w_k, w_v, w_o, core, n=N_TOKENS,
                     np_x=ml_dtypes.bfloat16, np_pt=ml_dtypes.bfloat16):
    """Host-side sharding/layout prep for one core. All fp32 numpy in."""
    b = core // 4
    g = core % 4
    hs = g * D_LOCAL
    scale = 1.0 / np.sqrt(D_K)
    qt = np.ascontiguousarray(Q[b].T).astype(np_x)
    kt = np.ascontiguousarray(K[b].T).astype(np_x)
    vt = np.ascontiguousarray(V[b].T).astype(np_x)
    wql = w_q[hs:hs + D_LOCAL] * scale
    wkl = w_k[hs:hs + D_LOCAL]
    # column order [q0 q1 | q2 k2 | k0 k1 | v] (see build_nc); q/k/v
    # packed into one tensor so the device loads them with one DMA
    wqkv = np.ascontiguousarray(
        np.concatenate([wql[0:128], wql[128:192], wkl[128:192], wkl[0:128],
                        w_v[hs:hs + D_LOCAL]], axis=0).T
    ).astype(np_x)
    # w_o row-chunks ([0:128] then [128:192] at partitions 0-63) and the
    # causal mask, packed into one [128, 2*768+128] tensor
    wo = w_o[:, hs:hs + D_LOCAL].T
    wocm = np.zeros((P, 2 * D_MODEL + P), dtype=np.float32)
    wocm[:, 0:D_MODEL] = wo[0:P]
    wocm[0:64, D_MODEL:2 * D_MODEL] = wo[P:D_LOCAL]
    wocm[:, 2 * D_MODEL:] = make_causal_mask_np(np.float32)
    return {"qt": qt, "kt": kt, "vt": vt,
            "wqkv": wqkv, "wocm": wocm.astype(np_x)}


_NC_CACHE = {}


def _get_nc(key, **kw):
    if key not in _NC_CACHE:
        _NC_CACHE[key] = build_nc(**kw)
    return _NC_CACHE[key]


KCFG = {"mm": "bf16", "dt_x": BF16, "dt_pt": BF16, "dt_acc": BF16,
        "np_x": ml_dtypes.bfloat16, "np_pt": ml_dtypes.bfloat16}


def kernel(Q, K, V, w_q, w_k, w_v, w_o):
    Q = np.asarray(Q, dtype=np.float32)
    K = np.asarray(K, dtype=np.float32)
    V = np.asarray(V, dtype=np.float32)
    w_q = np.asarray(w_q, dtype=np.float32)
    w_k = np.asarray(w_k, dtype=np.float32)
    w_v = np.asarray(w_v, dtype=np.float32)
    w_o = np.asarray(w_o, dtype=np.float32)

    nc = _get_nc((KCFG["mm"], str(KCFG["dt_x"])),
                 n=N_TOKENS, mm=KCFG["mm"], dt_x=KCFG["dt_x"],
                 dt_pt=KCFG["dt_pt"], dt_acc=KCFG["dt_acc"])
    in_maps = [
        prep_core_inputs(Q, K, V, w_q, w_k, w_v, w_o, c,
                         np_x=KCFG["np_x"], np_pt=KCFG["np_pt"])
        for c in range(N_CORES)
    ]
    res = bass_utils.run_bass_kernel_spmd(nc, in_maps,
                                          core_ids=list(range(N_CORES)))
    out = np.zeros((B, N_TOKENS, D_MODEL), dtype=np.float32)
    for c in range(N_CORES):
        out[c // 4] += np.asarray(res.results[c]["y"], dtype=np.float32)
    return out
